# Initial kernel scaffold
#
"""KWinnersCompetition forward kernel for 8 Trainium2 NeuronCores.

The reference's top-k mask only gates gradients (where(mask, x, stop_grad(x))
has forward value x), so the forward output is exactly:

    out[b, c, h, w] = relu(x[b, c, h, w] - mean_c' x[b, c', h, w])

Sharding: data-parallel over batch. 64 batches / 8 cores = 8 per core,
no communication.

Per-core layout (x shard [8, 512, 784] f32, C-major so HW is contiguous).
Channels are interleaved onto partitions as c = 4p + j (partition p,
free-dim j in 0..3) so every partition's DMA run is one contiguous
4*784*4 = 12.5 KB stretch of DRAM — both load and store are maximally
DMA-efficient.

Per batch:
  - DMA in xt [128p, 4j, 784w] (one 1.6 MB fully-contiguous DMA)
  - PE:  per 392-col half, 4 accumulating matmuls with a constant
    1/512 weight tile: m = (1/512) * sum_c x[c, :] broadcast to all
    128 partitions (1/512 = 2^-9 is exact in f32; f32 PSUM accumulate)
  - DVE: one tensor_sub per half with the mean AP broadcast over j:
    dt[:, j, half] = xt[:, j, half] - m
  - ACT: relu per half: ot = relu(dt)
  - DMA out [128, 4, 784] once per batch.
"""

import sys

if "/opt/trn_rl_repo" not in sys.path:
    sys.path.insert(0, "/opt/trn_rl_repo")

import numpy as np

B, C, H, W = 64, 512, 28, 28
HW = H * W              # 784
NCORES = 8
BPC = B // NCORES       # 8 batches per core
P = 128                 # partitions
J = C // P              # 4 channels interleaved per partition
HALF = HW // 2          # 392 (matmul free dim <= 512 / one PSUM bank)

_built = None


def _build():
    import concourse.bacc as bacc
    import concourse.bass as bass
    import concourse.tile as tile
    from concourse import mybir

    nc = bacc.Bacc("TRN2", target_bir_lowering=False, debug=False)
    x = nc.dram_tensor("x", [BPC, C, HW], mybir.dt.float32, kind="ExternalInput")
    y = nc.dram_tensor("y", [BPC, C, HW], mybir.dt.float32, kind="ExternalOutput")

    with tile.TileContext(nc) as tc:
        with (
            tc.tile_pool(name="singles", bufs=1) as singles,
            tc.tile_pool(name="xin", bufs=BPC) as xin,
            tc.tile_pool(name="diffs", bufs=4) as diffs,
            tc.tile_pool(name="outs", bufs=4) as outs,
            tc.tile_pool(name="means", bufs=4, space="PSUM") as means,
        ):
            wones = singles.tile([P, P], mybir.dt.float32)
            nc.vector.memset(wones, 1.0 / C)

            for b in range(BPC):
                xb = x[b].rearrange("(p j) w -> p j w", j=J)
                yb = y[b].rearrange("(p j) w -> p j w", j=J)

                xt = xin.tile([P, J, HW], mybir.dt.float32)
                nc.sync.dma_start(out=xt, in_=xb)

                dt = diffs.tile([P, J, HW], mybir.dt.float32)
                ot = outs.tile([P, J, HW], mybir.dt.float32)

                for h in range(2):
                    lo = h * HALF
                    hi = lo + HALF
                    m = means.tile([P, HALF], mybir.dt.float32)
                    for j in range(J):
                        nc.tensor.matmul(
                            m,
                            wones,
                            xt[:, j, lo:hi],
                            start=(j == 0),
                            stop=(j == J - 1),
                        )
                    # mean AP broadcast across the j dim (step 0)
                    map_ = m[:]
                    m_bcast = bass.AP(
                        tensor=map_.tensor,
                        offset=map_.offset,
                        ap=[map_.ap[0], [0, J], map_.ap[1]],
                    )
                    nc.vector.tensor_sub(dt[:, :, lo:hi], xt[:, :, lo:hi], m_bcast)
                    nc.scalar.activation(
                        ot[:, :, lo:hi],
                        dt[:, :, lo:hi],
                        func=mybir.ActivationFunctionType.Relu,
                    )
                nc.sync.dma_start(out=yb, in_=ot)

    nc.compile()
    return nc


def _get_nc():
    global _built
    if _built is None:
        _built = _build()
    return _built


def _shard(x_full):
    xf = np.ascontiguousarray(x_full.reshape(B, C, HW), dtype=np.float32)
    return [
        {"x": np.ascontiguousarray(xf[i * BPC : (i + 1) * BPC])}
        for i in range(NCORES)
    ]


def _run(in_maps, **kw):
    from concourse.bass_utils import run_bass_kernel_spmd

    return run_bass_kernel_spmd(_get_nc(), in_maps, list(range(NCORES)), **kw)


def kernel(x, k=None, **_unused):
    res = _run(_shard(np.asarray(x)))
    out = np.concatenate([res.results[i]["y"] for i in range(NCORES)], axis=0)
    return out.reshape(B, C, H, W).astype(np.float32)


if __name__ == "__main__":
    xs = np.random.randn(B, C, H, W).astype(np.float32)
    got = kernel(xs, 52)
    exp = np.maximum(xs - xs.mean(axis=1, keepdims=True), 0.0)
    err = np.abs(got - exp).max()
    print("abs err vs numpy:", err)



# revision 1
# speedup vs baseline: 1.3039x; 1.3039x over previous
"""KWinnersCompetition forward kernel for 8 Trainium2 NeuronCores.

The reference's top-k mask only gates gradients (where(mask, x, stop_grad(x))
has forward value x), so the forward output is exactly:

    out[b, c, h, w] = relu(x[b, c, h, w] - mean_c' x[b, c', h, w])

Sharding: data-parallel over batch. 64 batches / 8 cores = 8 per core,
no communication.

Per-core layout (x shard [8, 512, 784] f32, C-major so HW is contiguous).
Channels are interleaved onto partitions as c = 4p + j (partition p,
free-dim j in 0..3) so every partition's DMA run is one contiguous
4*784*4 = 12.5 KB stretch of DRAM — both load and store are maximally
DMA-efficient.

Per batch:
  - DMA in xt [128p, 4j, 784w] (one 1.6 MB fully-contiguous DMA)
  - PE:  per 392-col half, 4 accumulating matmuls with a constant
    1/512 weight tile: m = (1/512) * sum_c x[c, :] broadcast to all
    128 partitions (1/512 = 2^-9 is exact in f32; f32 PSUM accumulate)
  - DVE: one tensor_sub per half with the mean AP broadcast over j:
    dt[:, j, half] = xt[:, j, half] - m
  - ACT: relu per half: ot = relu(dt)
  - DMA out [128, 4, 784] once per batch.
"""

import sys

if "/opt/trn_rl_repo" not in sys.path:
    sys.path.insert(0, "/opt/trn_rl_repo")

import numpy as np

B, C, H, W = 64, 512, 28, 28
HW = H * W              # 784
NCORES = 8
BPC = B // NCORES       # 8 batches per core
P = 128                 # partitions
J = C // P              # 4 channels interleaved per partition
HALF = HW // 2          # 392 (matmul free dim <= 512 / one PSUM bank)

_built = None


def _build():
    import concourse.bacc as bacc
    import concourse.bass as bass
    import concourse.tile as tile
    from concourse import mybir

    nc = bacc.Bacc("TRN2", target_bir_lowering=False, debug=False)
    x = nc.dram_tensor("x", [BPC, C, HW], mybir.dt.float32, kind="ExternalInput")
    y = nc.dram_tensor("y", [BPC, C, HW], mybir.dt.float32, kind="ExternalOutput")

    with tile.TileContext(nc) as tc:
        with (
            tc.tile_pool(name="singles", bufs=1) as singles,
            tc.tile_pool(name="xin", bufs=BPC) as xin,
            tc.tile_pool(name="diffs", bufs=4) as diffs,
            tc.tile_pool(name="outs", bufs=4) as outs,
            tc.tile_pool(name="means", bufs=4, space="PSUM") as means,
        ):
            wones = singles.tile([P, P], mybir.dt.float32)
            nc.vector.memset(wones, 1.0 / C)

            for b in range(BPC):
                xb = x[b].rearrange("(p j) w -> p j w", j=J)
                yb = y[b].rearrange("(p j) w -> p j w", j=J)

                xt = xin.tile([P, J, HW], mybir.dt.float32)
                nc.sync.dma_start(out=xt, in_=xb)

                dt = diffs.tile([P, J, HW], mybir.dt.float32)
                ot = outs.tile([P, J, HW], mybir.dt.float32)

                for h in range(2):
                    lo = h * HALF
                    hi = lo + HALF
                    m = means.tile([P, HALF], mybir.dt.float32)
                    for j in range(J):
                        nc.tensor.matmul(
                            m,
                            wones,
                            xt[:, j, lo:hi],
                            start=(j == 0),
                            stop=(j == J - 1),
                        )
                    # mean AP broadcast across the j dim (step 0)
                    map_ = m[:]
                    m_bcast = bass.AP(
                        tensor=map_.tensor,
                        offset=map_.offset,
                        ap=[map_.ap[0], [0, J], map_.ap[1]],
                    )
                    nc.vector.tensor_sub(dt[:, :, lo:hi], xt[:, :, lo:hi], m_bcast)
                    nc.scalar.activation(
                        ot[:, :, lo:hi],
                        dt[:, :, lo:hi],
                        func=mybir.ActivationFunctionType.Relu,
                    )
                nc.sync.dma_start(out=yb, in_=ot)

    nc.compile()
    return nc


def _get_nc():
    global _built
    if _built is None:
        _built = _build()
    return _built


def _shard(x_full):
    xf = np.ascontiguousarray(x_full.reshape(B, C, HW), dtype=np.float32)
    return [
        {"x": np.ascontiguousarray(xf[i * BPC : (i + 1) * BPC])}
        for i in range(NCORES)
    ]


def _run(in_maps, **kw):
    from concourse.bass_utils import run_bass_kernel_spmd

    return run_bass_kernel_spmd(_get_nc(), in_maps, list(range(NCORES)), **kw)


def kernel(x, k=None, **_unused):
    res = _run(_shard(np.asarray(x)))
    out = np.concatenate([res.results[i]["y"] for i in range(NCORES)], axis=0)
    return out.reshape(B, C, H, W).astype(np.float32)


if __name__ == "__main__":
    xs = np.random.randn(B, C, H, W).astype(np.float32)
    got = kernel(xs, 52)
    exp = np.maximum(xs - xs.mean(axis=1, keepdims=True), 0.0)
    err = np.abs(got - exp).max()
    print("abs err vs numpy:", err)



# revision 3
# speedup vs baseline: 1.3665x; 1.0480x over previous
"""KWinnersCompetition forward kernel for 8 Trainium2 NeuronCores.

The reference's top-k mask only gates gradients (where(mask, x, stop_grad(x))
has forward value x), so the forward output is exactly:

    out[b, c, h, w] = relu(x[b, c, h, w] - mean_c' x[b, c', h, w])

Sharding: data-parallel over batch. 64 batches / 8 cores = 8 per core,
no communication.

Per-core layout (x shard [8, 512, 784] f32, C-major so HW is contiguous).
Channels are interleaved onto partitions as c = 4p + j (partition p,
free-dim j in 0..3) so every partition's DMA run is one contiguous
4*784*4 = 12.5 KB stretch of DRAM — both load and store are maximally
DMA-efficient.

DMA plan (the kernel is memory-bound; ~25.7 MB of mandatory HBM traffic
per core): loads are issued on the Sync engine's HWDGE ring and stores
on the Scalar engine's HWDGE ring. Two independent FIFO rings mean a
store waiting on compute can never head-of-line-block a later load
(with a single ring the baseline lost ~9 us to exactly that). All 8
batch loads are enqueued up front (xin pool holds the full shard).

Compute per batch:
  - PE:  per 392-col half, 4 accumulating fp32r matmuls with a constant
    1/512 weight tile: m = (1/512) * sum_c x[c, :] broadcast to all
    128 partitions. fp32r runs 1 cycle/row vs 4 for full fp32 and is
    far more than accurate enough for a mean (tolerance is 2e-2).
  - DVE: one tensor_sub per half with the mean AP broadcast over j.
  - ACT: relu per half, then the store dma_start for the batch is
    issued from the same (Scalar) engine right after its data is ready.
"""

import sys

if "/opt/trn_rl_repo" not in sys.path:
    sys.path.insert(0, "/opt/trn_rl_repo")

import numpy as np

B, C, H, W = 64, 512, 28, 28
HW = H * W              # 784
NCORES = 8
BPC = B // NCORES       # 8 batches per core
P = 128                 # partitions
J = C // P              # 4 channels interleaved per partition
HALF = HW // 2          # 392 (matmul free dim <= 512 / one PSUM bank)

_built = None


def _build():
    import concourse.bacc as bacc
    import concourse.bass as bass
    import concourse.tile as tile
    from concourse import mybir

    nc = bacc.Bacc("TRN2", target_bir_lowering=False, debug=False)
    x = nc.dram_tensor("x", [BPC, C, HW], mybir.dt.float32, kind="ExternalInput")
    y = nc.dram_tensor("y", [BPC, C, HW], mybir.dt.float32, kind="ExternalOutput")

    f32r = mybir.dt.float32r

    with tile.TileContext(nc) as tc:
        with (
            tc.tile_pool(name="singles", bufs=1) as singles,
            tc.tile_pool(name="xin", bufs=BPC) as xin,
            tc.tile_pool(name="diffs", bufs=3) as diffs,
            tc.tile_pool(name="outs", bufs=3) as outs,
            tc.tile_pool(name="means", bufs=4, space="PSUM") as means,
        ):
            wones = singles.tile([P, P], mybir.dt.float32)
            nc.vector.memset(wones, 1.0 / C)

            xts = []
            for b in range(BPC):
                xb = x[b].rearrange("(p j) w -> p j w", j=J)
                xt = xin.tile([P, J, HW], mybir.dt.float32)
                # loads: Sync HWDGE ring, all enqueued up front
                nc.sync.dma_start(out=xt, in_=xb)
                xts.append(xt)

            for b in range(BPC):
                yb = y[b].rearrange("(p j) w -> p j w", j=J)
                xt = xts[b]

                dt = diffs.tile([P, J, HW], mybir.dt.float32)
                ot = outs.tile([P, J, HW], mybir.dt.float32)

                for h in range(2):
                    lo = h * HALF
                    hi = lo + HALF
                    m = means.tile([P, HALF], mybir.dt.float32)
                    for j in range(J):
                        nc.tensor.matmul(
                            m,
                            wones,
                            xt[:, j, lo:hi],
                            start=(j == 0),
                            stop=(j == J - 1),
                        )
                    # mean AP broadcast across the j dim (step 0)
                    map_ = m[:]
                    m_bcast = bass.AP(
                        tensor=map_.tensor,
                        offset=map_.offset,
                        ap=[map_.ap[0], [0, J], map_.ap[1]],
                    )
                    nc.vector.tensor_sub(dt[:, :, lo:hi], xt[:, :, lo:hi], m_bcast)
                    nc.scalar.activation(
                        ot[:, :, lo:hi],
                        dt[:, :, lo:hi],
                        func=mybir.ActivationFunctionType.Relu,
                    )
                # stores: Scalar HWDGE ring (same engine as the relu, so the
                # issue happens immediately after the data is ready and can
                # never block the Sync ring's loads)
                nc.scalar.dma_start(out=yb, in_=ot)

    nc.compile()
    return nc


def _get_nc():
    global _built
    if _built is None:
        _built = _build()
    return _built


def _shard(x_full):
    xf = np.ascontiguousarray(x_full.reshape(B, C, HW), dtype=np.float32)
    return [
        {"x": np.ascontiguousarray(xf[i * BPC : (i + 1) * BPC])}
        for i in range(NCORES)
    ]


def _run(in_maps, **kw):
    from concourse.bass_utils import run_bass_kernel_spmd

    return run_bass_kernel_spmd(_get_nc(), in_maps, list(range(NCORES)), **kw)


def kernel(x, k=None, **_unused):
    res = _run(_shard(np.asarray(x)))
    out = np.concatenate([res.results[i]["y"] for i in range(NCORES)], axis=0)
    return out.reshape(B, C, H, W).astype(np.float32)


if __name__ == "__main__":
    xs = np.random.randn(B, C, H, W).astype(np.float32)
    got = kernel(xs, 52)
    exp = np.maximum(xs - xs.mean(axis=1, keepdims=True), 0.0)
    err = np.abs(got - exp).max()
    print("abs err vs numpy:", err)


# revision 6
# speedup vs baseline: 1.5538x; 1.1371x over previous
"""KWinnersCompetition forward kernel for 8 Trainium2 NeuronCores.

The reference's top-k mask only gates gradients (where(mask, x, stop_grad(x))
has forward value x), so the forward output is exactly:

    out[b, c, h, w] = relu(x[b, c, h, w] - mean_c' x[b, c', h, w])

Sharding: data-parallel over batch. 64 batches / 8 cores = 8 per core,
no communication.

Per-core layout (x shard [8, 512, 784] f32, C-major so HW is contiguous).
Channels are interleaved onto partitions as c = 4p + j (partition p,
free-dim j in 0..3) so every partition's DMA run is one contiguous
4*784*4 = 12.5 KB stretch of DRAM — both load and store are maximally
DMA-efficient.

DMA plan (the kernel is memory-bound; ~25.7 MB of mandatory HBM traffic
per core): loads are issued on the Sync engine's HWDGE ring and stores
on the Scalar engine's HWDGE ring. Two independent FIFO rings mean a
store waiting on compute can never head-of-line-block a later load
(with a single ring the baseline lost ~9 us to exactly that). All 8
batch loads are enqueued up front (xin pool holds the full shard).

Compute per batch:
  - DVE: one cast of the batch tile to bf16 (the mean input only; the
    subtract still uses the f32 data).
  - PE:  per 392-col half, 4 accumulating bf16 matmuls with a constant
    1/512 weight tile: m = (1/512) * sum_c x[c, :] broadcast to all
    128 partitions (f32 PSUM accumulate). bf16 runs 1 cycle/row vs 4+
    for full fp32 — with fp32 the PE was the pipeline pacemaker
    (~5.2 us/batch serial) and starved the store stream. A bf16-rounded
    mean is ~1e-4 accurate; tolerance is 2e-2.
  - DVE: one tensor_sub per half with the mean AP broadcast over j.
  - ACT: relu per half, then the store dma_start for the batch is
    issued from the same (Scalar) engine right after its data is ready.
"""

import sys

if "/opt/trn_rl_repo" not in sys.path:
    sys.path.insert(0, "/opt/trn_rl_repo")

import numpy as np

B, C, H, W = 64, 512, 28, 28
HW = H * W              # 784
NCORES = 8
BPC = B // NCORES       # 8 batches per core
P = 128                 # partitions
J = C // P              # 4 channels interleaved per partition
HALF = HW // 2          # 392 (matmul free dim <= 512 / one PSUM bank)

_built = None


def _build():
    import concourse.bacc as bacc
    import concourse.bass as bass
    import concourse.tile as tile
    from concourse import mybir

    nc = bacc.Bacc("TRN2", target_bir_lowering=False, debug=False)
    x = nc.dram_tensor("x", [BPC, C, HW], mybir.dt.float32, kind="ExternalInput")
    y = nc.dram_tensor("y", [BPC, C, HW], mybir.dt.float32, kind="ExternalOutput")

    with tile.TileContext(nc) as tc:
        with (
            tc.tile_pool(name="singles", bufs=1) as singles,
            tc.tile_pool(name="xin", bufs=BPC) as xin,
            tc.tile_pool(name="x16", bufs=2) as x16,
            tc.tile_pool(name="diffs", bufs=3) as diffs,
            tc.tile_pool(name="outs", bufs=3) as outs,
            tc.tile_pool(name="means", bufs=4, space="PSUM") as means,
        ):
            wones = singles.tile([P, P], mybir.dt.bfloat16)
            nc.vector.memset(wones, 1.0 / C)

            xts = []
            for b in range(BPC):
                xb = x[b].rearrange("(p j) w -> p j w", j=J)
                xt = xin.tile([P, J, HW], mybir.dt.float32)
                # loads: Sync HWDGE ring, all enqueued up front
                nc.sync.dma_start(out=xt, in_=xb)
                xts.append(xt)

            for b in range(BPC):
                yb = y[b].rearrange("(p j) w -> p j w", j=J)
                xt = xts[b]

                xb = x16.tile([P, J, HW], mybir.dt.bfloat16)
                nc.vector.tensor_copy(out=xb, in_=xt)

                dt = diffs.tile([P, J, HW], mybir.dt.float32)
                ot = outs.tile([P, J, HW], mybir.dt.float32)

                for h in range(2):
                    lo = h * HALF
                    hi = lo + HALF
                    m = means.tile([P, HALF], mybir.dt.float32)
                    for j in range(J):
                        nc.tensor.matmul(
                            m,
                            wones,
                            xb[:, j, lo:hi],
                            start=(j == 0),
                            stop=(j == J - 1),
                        )
                    # mean AP broadcast across the j dim (step 0)
                    map_ = m[:]
                    m_bcast = bass.AP(
                        tensor=map_.tensor,
                        offset=map_.offset,
                        ap=[map_.ap[0], [0, J], map_.ap[1]],
                    )
                    nc.vector.tensor_sub(dt[:, :, lo:hi], xt[:, :, lo:hi], m_bcast)
                    nc.scalar.activation(
                        ot[:, :, lo:hi],
                        dt[:, :, lo:hi],
                        func=mybir.ActivationFunctionType.Relu,
                    )
                # stores: Scalar HWDGE ring (same engine as the relu, so the
                # issue happens immediately after the data is ready and can
                # never block the Sync ring's loads)
                nc.scalar.dma_start(out=yb, in_=ot)

    nc.compile()
    return nc


def _get_nc():
    global _built
    if _built is None:
        _built = _build()
    return _built


def _shard(x_full):
    xf = np.ascontiguousarray(x_full.reshape(B, C, HW), dtype=np.float32)
    return [
        {"x": np.ascontiguousarray(xf[i * BPC : (i + 1) * BPC])}
        for i in range(NCORES)
    ]


def _run(in_maps, **kw):
    from concourse.bass_utils import run_bass_kernel_spmd

    return run_bass_kernel_spmd(_get_nc(), in_maps, list(range(NCORES)), **kw)


def kernel(x, k=None, **_unused):
    res = _run(_shard(np.asarray(x)))
    out = np.concatenate([res.results[i]["y"] for i in range(NCORES)], axis=0)
    return out.reshape(B, C, H, W).astype(np.float32)


if __name__ == "__main__":
    xs = np.random.randn(B, C, H, W).astype(np.float32)
    got = kernel(xs, 52)
    exp = np.maximum(xs - xs.mean(axis=1, keepdims=True), 0.0)
    err = np.abs(got - exp).max()
    print("abs err vs numpy:", err)
